# revision 17
# baseline (speedup 1.0000x reference)
"""Additive soft attention Trainium2 kernel.

Computation per batch b:
    attn_h = h @ W_h2attn.T + b_h2attn                       [ATT]
    dot    = tanh(proj_context[b] + attn_h)                  [S, ATT]
    scores = dot @ w_alpha + b_alpha                         [S]
    scores = where(mask, MIN_VALUE, scores)
    attn   = softmax(scores)                                 [S]
    wc     = attn @ context[b]                               [CTX]

Sharding: data-parallel over batch, 8 batches per core on 8 cores.

Per-core design (BL = 8 local batches, processed as 2 groups of 4 so the
context stream of group g overlaps the proj/compute of group g+1):
  - proj tiles are loaded 4-rows-per-partition packed ([128, 4*512] per
    512-s chunk -> 1MB contiguous DMAs) and transposed on the PE (fp32r
    transpose mode, 1.5 cyc/row) into ATT-on-partitions layout, so the
    attn_h broadcast-add fuses into the ACT tanh as a per-partition bias.
  - scores = sum_a w_alpha[a]*dotT[a, s] via PE matmuls (fp32r, full rate),
    accumulated in PSUM over the 4 ATT tiles.
  - softmax is batched over the 4 group rows with exp+sum fused in one
    ACT op (accum_out).
  - weighted context: per (batch, s-tile) PE matmul with the transposed
    attn column as stationary, context tiles loaded 4-row packed (2MB DMAs).
  - within each 512-s chunk the s order is permuted (s = 4p + j); mask in
    and attn out are fixed up with single strided DVE copies.
"""

import numpy as np

_B, _S, _RNN, _ATT, _CTX = 64, 2048, 1024, 512, 1024
_NCORES = 8
_BL = _B // _NCORES  # 8 batches per core
_MIN = -100000000.0

_NAT = _ATT // 128  # 4 a-tiles
_NKT = _RNN // 128  # 8 k-tiles
_NCH = 4            # s chunks of 512
_NJ = 4             # s rows packed per partition
_G = 2              # batch groups per core
_GB = _BL // _G     # batches per group

_CACHE = {}


def _build_nc(reps=1):
    import concourse.bacc as bacc
    import concourse.mybir as mybir
    import concourse.tile as tile

    f32 = mybir.dt.float32
    f32r = mybir.dt.float32r
    u8 = mybir.dt.uint8
    Tanh = mybir.ActivationFunctionType.Tanh
    Exp = mybir.ActivationFunctionType.Exp
    Ident = mybir.ActivationFunctionType.Identity
    AX = mybir.AxisListType.X
    Alu = mybir.AluOpType

    # Bacc (not raw Bass): its compile() runs move_matmul_waits_to_ldweights
    # + generate_event_semaphores, which walrus requires (max 1 wait/inst).
    nc = bacc.Bacc("TRN2", target_bir_lowering=False, debug=False)

    h_d = nc.dram_tensor("h", [_BL, _RNN], f32, kind="ExternalInput")
    pj_d = nc.dram_tensor("proj_context", [_BL, _S, _ATT], f32r, kind="ExternalInput")
    cx_d = nc.dram_tensor("context", [_BL, _S, _CTX], f32r, kind="ExternalInput")
    mk_d = nc.dram_tensor("mask", [_BL, _S], u8, kind="ExternalInput")
    W_d = nc.dram_tensor("W_h2attn", [_ATT, _RNN], f32, kind="ExternalInput")
    bh_d = nc.dram_tensor("b_h2attn", [_ATT], f32, kind="ExternalInput")
    wa_d = nc.dram_tensor("w_alpha", [_ATT], f32r, kind="ExternalInput")
    ba_d = nc.dram_tensor("b_alpha", [1], f32, kind="ExternalInput")

    wc_d = nc.dram_tensor("weighted_context", [_BL, _CTX], f32, kind="ExternalOutput")
    at_d = nc.dram_tensor("attn", [_BL, _S], f32, kind="ExternalOutput")

    ident_d = nc.inline_tensor(np.eye(128, dtype=np.float32), name="ident128")

    with tile.TileContext(nc) as tc:
        with (
            tc.tile_pool(name="const", bufs=1) as constp,
            tc.tile_pool(name="grouppool", bufs=2) as grouppool,
            tc.tile_pool(name="pjpool", bufs=3) as pjpool,
            tc.tile_pool(name="dotpool", bufs=3) as dotpool,
            tc.tile_pool(name="cxpool", bufs=3) as cxpool,
            tc.tile_pool(name="smallp", bufs=2) as smallp,
            tc.tile_pool(name="rowpool", bufs=2) as rowpool,
            tc.tile_pool(name="stage_ps", bufs=2, space="PSUM") as stage_ps,
            tc.tile_pool(name="scps", bufs=1, space="PSUM") as scps_pool,
            tc.tile_pool(name="wcps", bufs=1, space="PSUM") as wcps_pool,
        ):
            # ---------------- constants / setup ----------------
            ident = constp.tile([128, 128], f32)
            nc.sync.dma_start(ident[:], ident_d[:])
            identr = constp.tile([128, 128], f32r)
            nc.sync.dma_start(identr[:], ident_d[:].bitcast(f32r))

            wa_sb = constp.tile([128, _NAT], f32r)
            nc.sync.dma_start(wa_sb[:], wa_d[:].rearrange("(t p) -> p t", p=128))
            bh_sb = constp.tile([128, _NAT], f32)
            nc.sync.dma_start(bh_sb[:], bh_d[:].rearrange("(t p) -> p t", p=128))
            ba_sb = constp.tile([1, 1], f32)
            nc.sync.dma_start(ba_sb[:], ba_d[:].rearrange("(a b) -> a b", a=1))

            # h^T: [128k, kt*8 + b]
            h_sb = constp.tile([_BL, _RNN], f32)
            nc.sync.dma_start(h_sb[:], h_d[:])
            hT = constp.tile([128, _NKT * _BL], f32)
            for kt in range(_NKT):
                tp = stage_ps.tile([128, _BL], f32, tag="stage", name=f"tph{kt}")
                nc.tensor.transpose(
                    tp[:], h_sb[:, kt * 128 : (kt + 1) * 128], ident[0:_BL, 0:_BL]
                )
                nc.vector.tensor_copy(hT[:, kt * _BL : (kt + 1) * _BL], tp[:])

            # W^T: [128k, kt*512 + a]
            WT = constp.tile([128, _NKT * _ATT], f32)
            for at in range(_NAT):
                w_nat = smallp.tile([128, _RNN], f32, tag="wnat", name=f"wnat{at}")
                nc.sync.dma_start(w_nat[:], W_d[at * 128 : (at + 1) * 128, :])
                for kt in range(_NKT):
                    tpw = stage_ps.tile(
                        [128, 128], f32, tag="stage", name=f"tpw{at}_{kt}"
                    )
                    nc.tensor.transpose(
                        tpw[:], w_nat[:, kt * 128 : (kt + 1) * 128], ident[:]
                    )
                    nc.vector.tensor_copy(
                        WT[:, kt * _ATT + at * 128 : kt * _ATT + (at + 1) * 128],
                        tpw[:],
                    )

            # attn_h^T: [128a, at*8 + b]
            attn_hT = constp.tile([128, _NAT * _BL], f32)
            for at in range(_NAT):
                ah_ps = scps_pool.tile([128, _BL], f32, tag="sc", name=f"ahps{at}")
                for kt in range(_NKT):
                    nc.tensor.matmul(
                        ah_ps[:],
                        WT[:, kt * _ATT + at * 128 : kt * _ATT + (at + 1) * 128],
                        hT[:, kt * _BL : (kt + 1) * _BL],
                        start=(kt == 0),
                        stop=(kt == _NKT - 1),
                    )
                nc.scalar.activation(
                    attn_hT[:, at * _BL : (at + 1) * _BL],
                    ah_ps[:],
                    Ident,
                    bias=bh_sb[:, at : at + 1],
                    scale=1.0,
                )

            minval = constp.tile([_GB, _S], f32)
            nc.vector.memset(minval[:], _MIN)

            for _rep in range(reps):
                for g in range(_G):
                    # -------- group state --------
                    mask_g = smallp.tile(
                        [_GB, _S], u8, tag="wnat", name=f"maskg{_rep}_{g}"
                    )
                    nc.sync.dma_start(mask_g[:], mk_d[g * _GB : (g + 1) * _GB, :])
                    mask_p = grouppool.tile(
                        [_GB, _S], u8, tag="maskp", name=f"maskp{_rep}_{g}"
                    )
                    nc.vector.tensor_copy(
                        mask_p[:].rearrange(
                            "b (c j p) -> b c j p", c=_NCH, j=_NJ, p=128
                        ),
                        mask_g[:].rearrange(
                            "b (c p j) -> b c j p", c=_NCH, p=128, j=_NJ
                        ),
                    )
                    scores_g = grouppool.tile(
                        [_GB, _S], f32, tag="scores", name=f"scores{_rep}_{g}"
                    )
                    attn_g = grouppool.tile(
                        [_GB, _S], f32, tag="attng", name=f"attng{_rep}_{g}"
                    )
                    attnT_g = grouppool.tile(
                        [128, _NCH * _NJ * _GB], f32r, tag="attnT",
                        name=f"attnT{_rep}_{g}",
                    )
                    mx = grouppool.tile([_GB, 1], f32, tag="mx", name=f"mx{_rep}_{g}")
                    sume = grouppool.tile(
                        [_GB, 1], f32, tag="sume", name=f"sume{_rep}_{g}"
                    )
                    rsum = grouppool.tile(
                        [_GB, 1], f32, tag="rsum", name=f"rsum{_rep}_{g}"
                    )

                    # -------- phase A: scores --------
                    for bl in range(_GB):
                        b = g * _GB + bl
                        scrow = rowpool.tile(
                            [1, _S], f32, tag="scrow", name=f"scrow{_rep}_{b}"
                        )
                        for cp in range(_NCH // 2):
                            pjt = []
                            for half in range(2):
                                c = cp * 2 + half
                                pj = pjpool.tile(
                                    [128, _NJ, _ATT], f32r, tag="pj",
                                    name=f"pj{_rep}_{b}_{c}",
                                )
                                nc.sync.dma_start(
                                    pj[:],
                                    pj_d[b, c * 512 : (c + 1) * 512, :].rearrange(
                                        "(p j) a -> p j a", p=128, j=_NJ
                                    ),
                                )
                                pjt.append(pj)
                            sc = scps_pool.tile(
                                [1, 1024], f32, tag="sc", name=f"sc{_rep}_{b}_{cp}"
                            )
                            for at in range(_NAT):
                                stg = stage_ps.tile(
                                    [128, 1024], f32r, tag="stage",
                                    name=f"stg{_rep}_{b}_{cp}_{at}",
                                )
                                for half in range(2):
                                    for j in range(_NJ):
                                        nc.tensor.transpose(
                                            stg[
                                                :,
                                                half * 512 + j * 128 :
                                                half * 512 + (j + 1) * 128,
                                            ],
                                            pjt[half][:, j, at * 128 : (at + 1) * 128],
                                            identr[:],
                                        )
                                dotT = dotpool.tile(
                                    [128, 1024], f32r, tag="dot",
                                    name=f"dot{_rep}_{b}_{cp}_{at}",
                                )
                                nc.scalar.activation(
                                    dotT[:],
                                    stg[:],
                                    Tanh,
                                    bias=attn_hT[:, at * _BL + b : at * _BL + b + 1],
                                    scale=1.0,
                                )
                                for n in range(2):
                                    nc.tensor.matmul(
                                        sc[0:1, n * 512 : (n + 1) * 512],
                                        wa_sb[:, at : at + 1],
                                        dotT[:, n * 512 : (n + 1) * 512],
                                        start=(at == 0),
                                        stop=(at == _NAT - 1),
                                    )
                            nc.vector.tensor_scalar_add(
                                scrow[0:1, cp * 1024 : (cp + 1) * 1024],
                                sc[:],
                                ba_sb[0:1, 0:1],
                            )
                        nc.gpsimd.dma_start(scores_g[bl : bl + 1, :], scrow[:])

                    # -------- mask + softmax (4 rows at once) --------
                    nc.vector.copy_predicated(scores_g[:], mask_p[:], minval[:])
                    nc.vector.tensor_reduce(
                        mx[:], scores_g[:], axis=AX, op=Alu.max, negate=True
                    )
                    nc.scalar.activation(
                        attn_g[:], scores_g[:], Exp, bias=mx[:], scale=1.0,
                        accum_out=sume[:],
                    )
                    nc.vector.reciprocal(rsum[:], sume[:])
                    nc.vector.tensor_scalar_mul(attn_g[:], attn_g[:], rsum[:])

                    # un-permute attn (into the dead scores_g tile) and store
                    nc.vector.tensor_copy(
                        scores_g[:].rearrange(
                            "b (c p j) -> b c p j", c=_NCH, p=128, j=_NJ
                        ),
                        attn_g[:].rearrange(
                            "b (c j p) -> b c p j", c=_NCH, j=_NJ, p=128
                        ),
                    )
                    nc.gpsimd.dma_start(
                        at_d[g * _GB : (g + 1) * _GB, :], scores_g[:]
                    )

                    # attn^T columns for the weighted-context matmuls
                    for t in range(_NCH * _NJ):
                        tpa = stage_ps.tile(
                            [128, _GB], f32, tag="stage", name=f"tpa{_rep}_{g}_{t}"
                        )
                        nc.tensor.transpose(
                            tpa[:],
                            attn_g[:, t * 128 : (t + 1) * 128],
                            ident[0:_GB, 0:_GB],
                        )
                        nc.vector.tensor_copy(
                            attnT_g[:, t * _GB : (t + 1) * _GB], tpa[:]
                        )

                    # -------- phase B: weighted context --------
                    for bl in range(_GB):
                        b = g * _GB + bl
                        wcp = wcps_pool.tile(
                            [1, _CTX], f32, tag="wc", name=f"wcp{_rep}_{b}"
                        )
                        for c in range(_NCH):
                            cx = cxpool.tile(
                                [128, _NJ, _CTX], f32r, tag="cx",
                                name=f"cx{_rep}_{b}_{c}",
                            )
                            nc.sync.dma_start(
                                cx[:],
                                cx_d[b, c * 512 : (c + 1) * 512, :].rearrange(
                                    "(p j) d -> p j d", p=128, j=_NJ
                                ),
                            )
                            for j in range(_NJ):
                                col = (c * _NJ + j) * _GB + bl
                                for n in range(2):
                                    nc.tensor.matmul(
                                        wcp[0:1, n * 512 : (n + 1) * 512],
                                        attnT_g[:, col : col + 1],
                                        cx[:, j, n * 512 : (n + 1) * 512],
                                        start=(c == 0 and j == 0),
                                        stop=(c == _NCH - 1 and j == _NJ - 1),
                                    )
                        wcrow = rowpool.tile(
                            [1, _CTX], f32, tag="wcrow", name=f"wcrow{_rep}_{b}"
                        )
                        nc.vector.tensor_copy(wcrow[:], wcp[:])
                        nc.gpsimd.dma_start(wc_d[b : b + 1, :], wcrow[:])

    nc.compile()
    return nc


def _get_nc(reps=1):
    key = ("nc", reps)
    if key not in _CACHE:
        _CACHE[key] = _build_nc(reps)
    return _CACHE[key]


def make_in_maps(**inputs):
    """Shard the full inputs into per-core input maps."""
    h = np.ascontiguousarray(np.asarray(inputs["h"], np.float32))
    pj = np.ascontiguousarray(np.asarray(inputs["proj_context"], np.float32))
    cx = np.ascontiguousarray(np.asarray(inputs["context"], np.float32))
    mk = np.ascontiguousarray(np.asarray(inputs["mask"]).astype(np.uint8))
    W = np.ascontiguousarray(np.asarray(inputs["W_h2attn"], np.float32))
    bh = np.ascontiguousarray(np.asarray(inputs["b_h2attn"], np.float32))
    wa = np.ascontiguousarray(np.asarray(inputs["w_alpha"], np.float32))
    ba = np.asarray(inputs["b_alpha"], np.float32).reshape(1)

    in_maps = []
    for core in range(_NCORES):
        sl = slice(core * _BL, (core + 1) * _BL)
        in_maps.append(
            {
                "h": h[sl],
                "proj_context": pj[sl],
                "context": cx[sl],
                "mask": mk[sl],
                "W_h2attn": W,
                "b_h2attn": bh,
                "w_alpha": wa,
                "b_alpha": ba,
            }
        )
    return in_maps


def kernel(**inputs):
    from concourse.bass_utils import run_bass_kernel_spmd

    nc = _get_nc()
    in_maps = make_in_maps(**inputs)
    res = run_bass_kernel_spmd(nc, in_maps, core_ids=list(range(_NCORES))).results

    wc = np.concatenate([res[c]["weighted_context"] for c in range(_NCORES)], axis=0)
    attn = np.concatenate([res[c]["attn"] for c in range(_NCORES)], axis=0)
    return wc, attn


# revision 22
# speedup vs baseline: 1.3670x; 1.3670x over previous
"""Additive soft attention Trainium2 kernel.

Computation per batch b:
    attn_h = h @ W_h2attn.T + b_h2attn                       [ATT]
    dot    = tanh(proj_context[b] + attn_h)                  [S, ATT]
    scores = dot @ w_alpha + b_alpha                         [S]
    scores = where(mask, MIN_VALUE, scores)
    attn   = softmax(scores)                                 [S]
    wc     = attn @ context[b]                               [CTX]

Sharding: data-parallel over batch, 8 batches per core on 8 cores.

Per-core design (BL = 8 local batches, processed as 2 groups of 4 so the
context stream of group g overlaps the proj/compute of group g+1):
  - proj tiles are loaded 4-rows-per-partition packed ([128, 4*512] per
    512-s chunk -> 1MB contiguous DMAs) and transposed on the PE (fp32r
    transpose mode, 1.5 cyc/row) into ATT-on-partitions layout, so the
    attn_h broadcast-add fuses into the ACT tanh as a per-partition bias.
  - scores = sum_a w_alpha[a]*dotT[a, s] via PE matmuls (fp32r, full rate),
    accumulated in PSUM over the 4 ATT tiles.
  - softmax is batched over the 4 group rows with exp+sum fused in one
    ACT op (accum_out).
  - weighted context: per (batch, s-tile) PE matmul with the transposed
    attn column as stationary, context tiles loaded 4-row packed (2MB DMAs).
  - within each 512-s chunk the s order is permuted (s = 4p + j); mask in
    and attn out are fixed up with single strided DVE copies.
"""

import numpy as np

_B, _S, _RNN, _ATT, _CTX = 64, 2048, 1024, 512, 1024
_NCORES = 8
_BL = _B // _NCORES  # 8 batches per core
_MIN = -100000000.0

_NAT = _ATT // 128  # 4 a-tiles
_NKT = _RNN // 128  # 8 k-tiles
_NCH = 4            # s chunks of 512
_NJ = 4             # s rows packed per partition
_G = 2              # batch groups per core
_GB = _BL // _G     # batches per group

_CACHE = {}


def _build_nc(reps=1):
    import concourse.bacc as bacc
    import concourse.mybir as mybir
    import concourse.tile as tile

    f32 = mybir.dt.float32
    f32r = mybir.dt.float32r
    u8 = mybir.dt.uint8
    Tanh = mybir.ActivationFunctionType.Tanh
    Exp = mybir.ActivationFunctionType.Exp
    Ident = mybir.ActivationFunctionType.Identity
    AX = mybir.AxisListType.X
    Alu = mybir.AluOpType

    # Bacc (not raw Bass): its compile() runs move_matmul_waits_to_ldweights
    # + generate_event_semaphores, which walrus requires (max 1 wait/inst).
    nc = bacc.Bacc("TRN2", target_bir_lowering=False, debug=False)

    h_d = nc.dram_tensor("h", [_BL, _RNN], f32, kind="ExternalInput")
    pj_d = nc.dram_tensor("proj_context", [_BL, _S, _ATT], f32r, kind="ExternalInput")
    cx_d = nc.dram_tensor("context", [_BL, _S, _CTX], f32r, kind="ExternalInput")
    mk_d = nc.dram_tensor("mask", [_BL, _S], u8, kind="ExternalInput")
    W_d = nc.dram_tensor("W_h2attn", [_ATT, _RNN], f32, kind="ExternalInput")
    bh_d = nc.dram_tensor("b_h2attn", [_ATT], f32, kind="ExternalInput")
    wa_d = nc.dram_tensor("w_alpha", [_ATT], f32r, kind="ExternalInput")
    ba_d = nc.dram_tensor("b_alpha", [1], f32, kind="ExternalInput")

    wc_d = nc.dram_tensor("weighted_context", [_BL, _CTX], f32, kind="ExternalOutput")
    at_d = nc.dram_tensor("attn", [_BL, _S], f32, kind="ExternalOutput")

    ident_d = nc.inline_tensor(np.eye(128, dtype=np.float32), name="ident128")

    with tile.TileContext(nc) as tc:
        with (
            tc.tile_pool(name="const", bufs=1) as constp,
            tc.tile_pool(name="grouppool", bufs=2) as grouppool,
            tc.tile_pool(name="pjpool", bufs=5) as pjpool,
            tc.tile_pool(name="dotpool", bufs=2) as dotpool,
            tc.tile_pool(name="cxpool", bufs=3) as cxpool,
            tc.tile_pool(name="smallp", bufs=2) as smallp,
            tc.tile_pool(name="rowpool", bufs=2) as rowpool,
            tc.tile_pool(name="stage_ps", bufs=2, space="PSUM") as stage_ps,
            tc.tile_pool(name="scps", bufs=1, space="PSUM") as scps_pool,
            tc.tile_pool(name="wcps", bufs=1, space="PSUM") as wcps_pool,
        ):
            # ---------------- constants / setup ----------------
            ident = constp.tile([128, 128], f32)
            nc.sync.dma_start(ident[:], ident_d[:])
            identr = constp.tile([128, 128], f32r)
            nc.sync.dma_start(identr[:], ident_d[:].bitcast(f32r))

            wa_sb = constp.tile([128, _NAT], f32r)
            nc.sync.dma_start(wa_sb[:], wa_d[:].rearrange("(t p) -> p t", p=128))
            bh_sb = constp.tile([128, _NAT], f32)
            nc.sync.dma_start(bh_sb[:], bh_d[:].rearrange("(t p) -> p t", p=128))
            ba_sb = constp.tile([1, 1], f32)
            nc.sync.dma_start(ba_sb[:], ba_d[:].rearrange("(a b) -> a b", a=1))

            # h^T: [128k, kt*8 + b]
            h_sb = constp.tile([_BL, _RNN], f32)
            nc.sync.dma_start(h_sb[:], h_d[:])
            hT = constp.tile([128, _NKT * _BL], f32)
            for kt in range(_NKT):
                tp = stage_ps.tile([128, _BL], f32, tag="stage", name=f"tph{kt}")
                nc.tensor.transpose(
                    tp[:], h_sb[:, kt * 128 : (kt + 1) * 128], ident[0:_BL, 0:_BL]
                )
                nc.vector.tensor_copy(hT[:, kt * _BL : (kt + 1) * _BL], tp[:])

            # W^T: [128k, kt*512 + a]
            WT = constp.tile([128, _NKT * _ATT], f32)
            for at in range(_NAT):
                w_nat = smallp.tile([128, _RNN], f32, tag="wnat", name=f"wnat{at}")
                nc.sync.dma_start(w_nat[:], W_d[at * 128 : (at + 1) * 128, :])
                for kt in range(_NKT):
                    tpw = stage_ps.tile(
                        [128, 128], f32, tag="stage", name=f"tpw{at}_{kt}"
                    )
                    nc.tensor.transpose(
                        tpw[:], w_nat[:, kt * 128 : (kt + 1) * 128], ident[:]
                    )
                    nc.vector.tensor_copy(
                        WT[:, kt * _ATT + at * 128 : kt * _ATT + (at + 1) * 128],
                        tpw[:],
                    )

            # attn_h^T: [128a, at*8 + b]
            attn_hT = constp.tile([128, _NAT * _BL], f32)
            for at in range(_NAT):
                ah_ps = scps_pool.tile([128, _BL], f32, tag="sc", name=f"ahps{at}")
                for kt in range(_NKT):
                    nc.tensor.matmul(
                        ah_ps[:],
                        WT[:, kt * _ATT + at * 128 : kt * _ATT + (at + 1) * 128],
                        hT[:, kt * _BL : (kt + 1) * _BL],
                        start=(kt == 0),
                        stop=(kt == _NKT - 1),
                    )
                nc.scalar.activation(
                    attn_hT[:, at * _BL : (at + 1) * _BL],
                    ah_ps[:],
                    Ident,
                    bias=bh_sb[:, at : at + 1],
                    scale=1.0,
                )

            minval = constp.tile([_GB, _S], f32)
            nc.vector.memset(minval[:], _MIN)

            def group_state(_rep, g):
                gs = {}
                mask_g = smallp.tile(
                    [_GB, _S], u8, tag="wnat", name=f"maskg{_rep}_{g}"
                )
                nc.sync.dma_start(mask_g[:], mk_d[g * _GB : (g + 1) * _GB, :])
                gs["mask_p"] = grouppool.tile(
                    [_GB, _S], u8, tag="maskp", name=f"maskp{_rep}_{g}"
                )
                nc.vector.tensor_copy(
                    gs["mask_p"][:].rearrange(
                        "b (c j p) -> b c j p", c=_NCH, j=_NJ, p=128
                    ),
                    mask_g[:].rearrange(
                        "b (c p j) -> b c j p", c=_NCH, p=128, j=_NJ
                    ),
                )
                gs["scores"] = grouppool.tile(
                    [_GB, _S], f32, tag="scores", name=f"scores{_rep}_{g}"
                )
                gs["attn"] = grouppool.tile(
                    [_GB, _S], f32, tag="attng", name=f"attng{_rep}_{g}"
                )
                gs["attnT"] = grouppool.tile(
                    [128, _NCH * _NJ * _GB], f32r, tag="attnT",
                    name=f"attnT{_rep}_{g}",
                )
                gs["mx"] = grouppool.tile(
                    [_GB, 1], f32, tag="mx", name=f"mx{_rep}_{g}"
                )
                gs["sume"] = grouppool.tile(
                    [_GB, 1], f32, tag="sume", name=f"sume{_rep}_{g}"
                )
                gs["rsum"] = grouppool.tile(
                    [_GB, 1], f32, tag="rsum", name=f"rsum{_rep}_{g}"
                )
                return gs

            def emit_A_batch(gs, _rep, g, bl):
                b = g * _GB + bl
                scrow = rowpool.tile(
                    [1, _S], f32, tag="scrow", name=f"scrow{_rep}_{b}", bufs=1
                )
                for cp in range(_NCH // 2):
                    pjt = []
                    for half in range(2):
                        c = cp * 2 + half
                        pj = pjpool.tile(
                            [128, _NJ, _ATT], f32r, tag="pj",
                            name=f"pj{_rep}_{b}_{c}",
                        )
                        nc.sync.dma_start(
                            pj[:],
                            pj_d[b, c * 512 : (c + 1) * 512, :].rearrange(
                                "(p j) a -> p j a", p=128, j=_NJ
                            ),
                        )
                        pjt.append(pj)
                    sc = scps_pool.tile(
                        [1, 1024], f32, tag="sc", name=f"sc{_rep}_{b}_{cp}"
                    )
                    for at in range(_NAT):
                        stg = stage_ps.tile(
                            [128, 1024], f32r, tag="stage",
                            name=f"stg{_rep}_{b}_{cp}_{at}",
                        )
                        for half in range(2):
                            for j in range(_NJ):
                                nc.tensor.transpose(
                                    stg[
                                        :,
                                        half * 512 + j * 128 :
                                        half * 512 + (j + 1) * 128,
                                    ],
                                    pjt[half][:, j, at * 128 : (at + 1) * 128],
                                    identr[:],
                                )
                        dotT = dotpool.tile(
                            [128, 1024], f32r, tag="dot",
                            name=f"dot{_rep}_{b}_{cp}_{at}",
                        )
                        nc.scalar.activation(
                            dotT[:],
                            stg[:],
                            Tanh,
                            bias=attn_hT[:, at * _BL + b : at * _BL + b + 1],
                            scale=1.0,
                        )
                        for n in range(2):
                            nc.tensor.matmul(
                                sc[0:1, n * 512 : (n + 1) * 512],
                                wa_sb[:, at : at + 1],
                                dotT[:, n * 512 : (n + 1) * 512],
                                start=(at == 0),
                                stop=(at == _NAT - 1),
                            )
                    nc.vector.tensor_scalar_add(
                        scrow[0:1, cp * 1024 : (cp + 1) * 1024],
                        sc[:],
                        ba_sb[0:1, 0:1],
                    )
                nc.gpsimd.dma_start(gs["scores"][bl : bl + 1, :], scrow[:])

            def emit_smx_chain(gs, _rep, g):
                scores_g = gs["scores"]
                attn_g = gs["attn"]
                nc.vector.copy_predicated(scores_g[:], gs["mask_p"][:], minval[:])
                nc.vector.tensor_reduce(
                    gs["mx"][:], scores_g[:], axis=AX, op=Alu.max, negate=True
                )
                nc.scalar.activation(
                    attn_g[:], scores_g[:], Exp, bias=gs["mx"][:], scale=1.0,
                    accum_out=gs["sume"][:],
                )
                nc.vector.reciprocal(gs["rsum"][:], gs["sume"][:])
                nc.vector.tensor_scalar_mul(attn_g[:], attn_g[:], gs["rsum"][:])

                # un-permute attn (into the dead scores tile); stored later
                nc.vector.tensor_copy(
                    scores_g[:].rearrange(
                        "b (c p j) -> b c p j", c=_NCH, p=128, j=_NJ
                    ),
                    attn_g[:].rearrange(
                        "b (c j p) -> b c p j", c=_NCH, j=_NJ, p=128
                    ),
                )

            def emit_attn_out(gs, _rep, g):
                nc.gpsimd.dma_start(
                    at_d[g * _GB : (g + 1) * _GB, :], gs["scores"][:]
                )

            def emit_attnT(gs, _rep, g):
                # attn^T columns for the weighted-context matmuls
                for t in range(_NCH * _NJ):
                    tpa = stage_ps.tile(
                        [128, _GB], f32, tag="stage", name=f"tpa{_rep}_{g}_{t}"
                    )
                    nc.tensor.transpose(
                        tpa[:],
                        gs["attn"][:, t * 128 : (t + 1) * 128],
                        ident[0:_GB, 0:_GB],
                    )
                    nc.vector.tensor_copy(
                        gs["attnT"][:, t * _GB : (t + 1) * _GB], tpa[:]
                    )

            pending_wcout = []

            def flush_wcout():
                while pending_wcout:
                    b, wcrow = pending_wcout.pop(0)
                    nc.gpsimd.dma_start(wc_d[b : b + 1, :], wcrow[:])

            def emit_B_batch(gs, _rep, g, bl):
                b = g * _GB + bl
                wcp = wcps_pool.tile(
                    [1, _CTX], f32, tag="wc", name=f"wcp{_rep}_{b}"
                )
                for c in range(_NCH):
                    cx = cxpool.tile(
                        [128, _NJ, _CTX], f32r, tag="cx",
                        name=f"cx{_rep}_{b}_{c}",
                    )
                    nc.gpsimd.dma_start(
                        cx[:],
                        cx_d[b, c * 512 : (c + 1) * 512, :].rearrange(
                            "(p j) d -> p j d", p=128, j=_NJ
                        ),
                    )
                    if c == _NCH - 1:
                        # the previous batch's wc row is ready by now; emitting
                        # it here keeps it from head-of-line-blocking this
                        # batch's context prefetch on the Pool DMA queue
                        flush_wcout()
                    for j in range(_NJ):
                        col = (c * _NJ + j) * _GB + bl
                        for n in range(2):
                            nc.tensor.matmul(
                                wcp[0:1, n * 512 : (n + 1) * 512],
                                gs["attnT"][:, col : col + 1],
                                cx[:, j, n * 512 : (n + 1) * 512],
                                start=(c == 0 and j == 0),
                                stop=(c == _NCH - 1 and j == _NJ - 1),
                            )
                wcrow = rowpool.tile(
                    [1, _CTX], f32, tag="wcrow", name=f"wcrow{_rep}_{b}"
                )
                nc.vector.tensor_copy(wcrow[:], wcp[:])
                pending_wcout.append((b, wcrow))

            # Software pipeline over (rep, group) units:
            #   A(first); smx(first); then per unit: interleave B(prev) with
            #   A(cur) at batch granularity (attnT/attn-out of prev slotted
            #   between batches); smx(cur); finally drain B(last).
            units = [(r, g) for r in range(reps) for g in range(_G)]
            prev = None
            prev_gs = None
            for unit in units:
                r, g = unit
                gs = group_state(r, g)
                for bl in range(_GB):
                    emit_A_batch(gs, r, g, bl)
                    if prev is not None:
                        if bl == 0:
                            emit_attnT(prev_gs, prev[0], prev[1])
                        if bl == 1:
                            emit_attn_out(prev_gs, prev[0], prev[1])
                        emit_B_batch(prev_gs, prev[0], prev[1], bl)
                emit_smx_chain(gs, r, g)
                prev, prev_gs = unit, gs
            emit_attnT(prev_gs, prev[0], prev[1])
            emit_attn_out(prev_gs, prev[0], prev[1])
            for bl in range(_GB):
                emit_B_batch(prev_gs, prev[0], prev[1], bl)
            flush_wcout()

    nc.compile()
    return nc


def _get_nc(reps=1):
    key = ("nc", reps)
    if key not in _CACHE:
        _CACHE[key] = _build_nc(reps)
    return _CACHE[key]


def make_in_maps(**inputs):
    """Shard the full inputs into per-core input maps."""
    h = np.ascontiguousarray(np.asarray(inputs["h"], np.float32))
    pj = np.ascontiguousarray(np.asarray(inputs["proj_context"], np.float32))
    cx = np.ascontiguousarray(np.asarray(inputs["context"], np.float32))
    mk = np.ascontiguousarray(np.asarray(inputs["mask"]).astype(np.uint8))
    W = np.ascontiguousarray(np.asarray(inputs["W_h2attn"], np.float32))
    bh = np.ascontiguousarray(np.asarray(inputs["b_h2attn"], np.float32))
    wa = np.ascontiguousarray(np.asarray(inputs["w_alpha"], np.float32))
    ba = np.asarray(inputs["b_alpha"], np.float32).reshape(1)

    in_maps = []
    for core in range(_NCORES):
        sl = slice(core * _BL, (core + 1) * _BL)
        in_maps.append(
            {
                "h": h[sl],
                "proj_context": pj[sl],
                "context": cx[sl],
                "mask": mk[sl],
                "W_h2attn": W,
                "b_h2attn": bh,
                "w_alpha": wa,
                "b_alpha": ba,
            }
        )
    return in_maps


def kernel(**inputs):
    from concourse.bass_utils import run_bass_kernel_spmd

    nc = _get_nc()
    in_maps = make_in_maps(**inputs)
    res = run_bass_kernel_spmd(nc, in_maps, core_ids=list(range(_NCORES))).results

    wc = np.concatenate([res[c]["weighted_context"] for c in range(_NCORES)], axis=0)
    attn = np.concatenate([res[c]["attn"] for c in range(_NCORES)], axis=0)
    return wc, attn


# revision 25
# speedup vs baseline: 1.3863x; 1.0142x over previous
"""Additive soft attention Trainium2 kernel.

Computation per batch b:
    attn_h = h @ W_h2attn.T + b_h2attn                       [ATT]
    dot    = tanh(proj_context[b] + attn_h)                  [S, ATT]
    scores = dot @ w_alpha + b_alpha                         [S]
    scores = where(mask, MIN_VALUE, scores)
    attn   = softmax(scores)                                 [S]
    wc     = attn @ context[b]                               [CTX]

Sharding: data-parallel over batch, 8 batches per core on 8 cores.

Per-core design (BL = 8 local batches, processed as 2 groups of 4 so the
context stream of group g overlaps the proj/compute of group g+1):
  - proj tiles are loaded 4-rows-per-partition packed ([128, 4*512] per
    512-s chunk -> 1MB contiguous DMAs) and transposed on the PE (fp32r
    transpose mode, 1.5 cyc/row) into ATT-on-partitions layout, so the
    attn_h broadcast-add fuses into the ACT tanh as a per-partition bias.
  - scores = sum_a w_alpha[a]*dotT[a, s] via PE matmuls (fp32r, full rate),
    accumulated in PSUM over the 4 ATT tiles.
  - softmax is batched over the 4 group rows with exp+sum fused in one
    ACT op (accum_out).
  - weighted context: per (batch, s-tile) PE matmul with the transposed
    attn column as stationary, context tiles loaded 4-row packed (2MB DMAs).
  - within each 512-s chunk the s order is permuted (s = 4p + j); mask in
    and attn out are fixed up with single strided DVE copies.
"""

import os
import sys

import numpy as np


def _ensure_concourse():
    try:
        import concourse.bass  # noqa: F401
        return
    except ImportError:
        pass
    for p in ("/opt/trn_rl_repo", "/root/.axon_site/_ro/trn_rl_repo"):
        if os.path.isdir(p) and p not in sys.path:
            sys.path.insert(0, p)
    import concourse.bass  # noqa: F401


_B, _S, _RNN, _ATT, _CTX = 64, 2048, 1024, 512, 1024
_NCORES = 8
_BL = _B // _NCORES  # 8 batches per core
_MIN = -100000000.0

_NAT = _ATT // 128  # 4 a-tiles
_NKT = _RNN // 128  # 8 k-tiles
_NCH = 4            # s chunks of 512
_NJ = 4             # s rows packed per partition
_G = 2              # batch groups per core
_GB = _BL // _G     # batches per group

_CACHE = {}


def _build_nc(reps=1):
    _ensure_concourse()
    import concourse.bacc as bacc
    import concourse.mybir as mybir
    import concourse.tile as tile

    f32 = mybir.dt.float32
    f32r = mybir.dt.float32r
    u8 = mybir.dt.uint8
    Tanh = mybir.ActivationFunctionType.Tanh
    Exp = mybir.ActivationFunctionType.Exp
    Ident = mybir.ActivationFunctionType.Identity
    AX = mybir.AxisListType.X
    Alu = mybir.AluOpType

    # Bacc (not raw Bass): its compile() runs move_matmul_waits_to_ldweights
    # + generate_event_semaphores, which walrus requires (max 1 wait/inst).
    nc = bacc.Bacc("TRN2", target_bir_lowering=False, debug=False)

    h_d = nc.dram_tensor("h", [_BL, _RNN], f32, kind="ExternalInput")
    pj_d = nc.dram_tensor("proj_context", [_BL, _S, _ATT], f32r, kind="ExternalInput")
    cx_d = nc.dram_tensor("context", [_BL, _S, _CTX], f32r, kind="ExternalInput")
    mk_d = nc.dram_tensor("mask", [_BL, _S], u8, kind="ExternalInput")
    W_d = nc.dram_tensor("W_h2attn", [_ATT, _RNN], f32, kind="ExternalInput")
    bh_d = nc.dram_tensor("b_h2attn", [_ATT], f32, kind="ExternalInput")
    wa_d = nc.dram_tensor("w_alpha", [_ATT], f32r, kind="ExternalInput")
    ba_d = nc.dram_tensor("b_alpha", [1], f32, kind="ExternalInput")

    wc_d = nc.dram_tensor("weighted_context", [_BL, _CTX], f32, kind="ExternalOutput")
    at_d = nc.dram_tensor("attn", [_BL, _S], f32, kind="ExternalOutput")

    ident_d = nc.inline_tensor(np.eye(128, dtype=np.float32), name="ident128")

    with tile.TileContext(nc) as tc:
        with (
            tc.tile_pool(name="const", bufs=1) as constp,
            tc.tile_pool(name="grouppool", bufs=2) as grouppool,
            tc.tile_pool(name="pjpool", bufs=5) as pjpool,
            tc.tile_pool(name="dotpool", bufs=2) as dotpool,
            tc.tile_pool(name="cxpool", bufs=3) as cxpool,
            tc.tile_pool(name="smallp", bufs=2) as smallp,
            tc.tile_pool(name="rowpool", bufs=2) as rowpool,
            tc.tile_pool(name="stage_ps", bufs=2, space="PSUM") as stage_ps,
            tc.tile_pool(name="scps", bufs=1, space="PSUM") as scps_pool,
            tc.tile_pool(name="wcps", bufs=1, space="PSUM") as wcps_pool,
        ):
            # ---------------- constants / setup ----------------
            ident = constp.tile([128, 128], f32)
            nc.sync.dma_start(ident[:], ident_d[:])
            identr = constp.tile([128, 128], f32r)
            nc.sync.dma_start(identr[:], ident_d[:].bitcast(f32r))

            wa_sb = constp.tile([128, _NAT], f32r)
            nc.sync.dma_start(wa_sb[:], wa_d[:].rearrange("(t p) -> p t", p=128))
            bh_sb = constp.tile([128, _NAT], f32)
            nc.sync.dma_start(bh_sb[:], bh_d[:].rearrange("(t p) -> p t", p=128))
            ba_sb = constp.tile([1, 1], f32)
            nc.sync.dma_start(ba_sb[:], ba_d[:].rearrange("(a b) -> a b", a=1))

            # h^T: [128k, kt*8 + b]
            h_sb = constp.tile([_BL, _RNN], f32)
            nc.sync.dma_start(h_sb[:], h_d[:])
            hT = constp.tile([128, _NKT * _BL], f32)
            for kt in range(_NKT):
                tp = stage_ps.tile([128, _BL], f32, tag="stage", name=f"tph{kt}")
                nc.tensor.transpose(
                    tp[:], h_sb[:, kt * 128 : (kt + 1) * 128], ident[0:_BL, 0:_BL]
                )
                nc.vector.tensor_copy(hT[:, kt * _BL : (kt + 1) * _BL], tp[:])

            # W^T: [128k, kt*512 + a]
            WT = constp.tile([128, _NKT * _ATT], f32)
            for at in range(_NAT):
                w_nat = smallp.tile([128, _RNN], f32, tag="wnat", name=f"wnat{at}")
                nc.sync.dma_start(w_nat[:], W_d[at * 128 : (at + 1) * 128, :])
                for kt in range(_NKT):
                    tpw = stage_ps.tile(
                        [128, 128], f32, tag="stage", name=f"tpw{at}_{kt}"
                    )
                    nc.tensor.transpose(
                        tpw[:], w_nat[:, kt * 128 : (kt + 1) * 128], ident[:]
                    )
                    nc.vector.tensor_copy(
                        WT[:, kt * _ATT + at * 128 : kt * _ATT + (at + 1) * 128],
                        tpw[:],
                    )

            # attn_h^T: [128a, at*8 + b]
            attn_hT = constp.tile([128, _NAT * _BL], f32)
            for at in range(_NAT):
                ah_ps = scps_pool.tile([128, _BL], f32, tag="sc", name=f"ahps{at}")
                for kt in range(_NKT):
                    nc.tensor.matmul(
                        ah_ps[:],
                        WT[:, kt * _ATT + at * 128 : kt * _ATT + (at + 1) * 128],
                        hT[:, kt * _BL : (kt + 1) * _BL],
                        start=(kt == 0),
                        stop=(kt == _NKT - 1),
                    )
                nc.scalar.activation(
                    attn_hT[:, at * _BL : (at + 1) * _BL],
                    ah_ps[:],
                    Ident,
                    bias=bh_sb[:, at : at + 1],
                    scale=1.0,
                )

            minval = constp.tile([_GB, _S], f32)
            nc.vector.memset(minval[:], _MIN)

            def group_state(_rep, g):
                gs = {}
                mask_g = smallp.tile(
                    [_GB, _S], u8, tag="wnat", name=f"maskg{_rep}_{g}"
                )
                nc.sync.dma_start(mask_g[:], mk_d[g * _GB : (g + 1) * _GB, :])
                gs["mask_p"] = grouppool.tile(
                    [_GB, _S], u8, tag="maskp", name=f"maskp{_rep}_{g}"
                )
                nc.vector.tensor_copy(
                    gs["mask_p"][:].rearrange(
                        "b (c j p) -> b c j p", c=_NCH, j=_NJ, p=128
                    ),
                    mask_g[:].rearrange(
                        "b (c p j) -> b c j p", c=_NCH, p=128, j=_NJ
                    ),
                )
                gs["scores"] = grouppool.tile(
                    [_GB, _S], f32, tag="scores", name=f"scores{_rep}_{g}"
                )
                gs["attn"] = grouppool.tile(
                    [_GB, _S], f32, tag="attng", name=f"attng{_rep}_{g}"
                )
                gs["attnT"] = grouppool.tile(
                    [128, _NCH * _NJ * _GB], f32r, tag="attnT",
                    name=f"attnT{_rep}_{g}",
                )
                gs["mx"] = grouppool.tile(
                    [_GB, 1], f32, tag="mx", name=f"mx{_rep}_{g}"
                )
                gs["sume"] = grouppool.tile(
                    [_GB, 1], f32, tag="sume", name=f"sume{_rep}_{g}"
                )
                gs["rsum"] = grouppool.tile(
                    [_GB, 1], f32, tag="rsum", name=f"rsum{_rep}_{g}"
                )
                return gs

            def emit_A_batch(gs, _rep, g, bl):
                b = g * _GB + bl
                scrow = rowpool.tile(
                    [1, _S], f32, tag="scrow", name=f"scrow{_rep}_{b}", bufs=1
                )
                for cp in range(_NCH // 2):
                    pjt = []
                    for half in range(2):
                        c = cp * 2 + half
                        pj = pjpool.tile(
                            [128, _NJ, _ATT], f32r, tag="pj",
                            name=f"pj{_rep}_{b}_{c}",
                        )
                        nc.sync.dma_start(
                            pj[:],
                            pj_d[b, c * 512 : (c + 1) * 512, :].rearrange(
                                "(p j) a -> p j a", p=128, j=_NJ
                            ),
                        )
                        pjt.append(pj)
                    sc = scps_pool.tile(
                        [1, 1024], f32, tag="sc", name=f"sc{_rep}_{b}_{cp}"
                    )
                    for at in range(_NAT):
                        stg = stage_ps.tile(
                            [128, 1024], f32r, tag="stage",
                            name=f"stg{_rep}_{b}_{cp}_{at}",
                        )
                        for half in range(2):
                            for j in range(_NJ):
                                nc.tensor.transpose(
                                    stg[
                                        :,
                                        half * 512 + j * 128 :
                                        half * 512 + (j + 1) * 128,
                                    ],
                                    pjt[half][:, j, at * 128 : (at + 1) * 128],
                                    identr[:],
                                )
                        dotT = dotpool.tile(
                            [128, 1024], f32r, tag="dot",
                            name=f"dot{_rep}_{b}_{cp}_{at}",
                        )
                        nc.scalar.activation(
                            dotT[:],
                            stg[:],
                            Tanh,
                            bias=attn_hT[:, at * _BL + b : at * _BL + b + 1],
                            scale=1.0,
                        )
                        for n in range(2):
                            nc.tensor.matmul(
                                sc[0:1, n * 512 : (n + 1) * 512],
                                wa_sb[:, at : at + 1],
                                dotT[:, n * 512 : (n + 1) * 512],
                                start=(at == 0),
                                stop=(at == _NAT - 1),
                            )
                    nc.vector.tensor_scalar_add(
                        scrow[0:1, cp * 1024 : (cp + 1) * 1024],
                        sc[:],
                        ba_sb[0:1, 0:1],
                    )
                nc.gpsimd.dma_start(gs["scores"][bl : bl + 1, :], scrow[:])

            def emit_smx_chain(gs, _rep, g):
                scores_g = gs["scores"]
                attn_g = gs["attn"]
                nc.vector.copy_predicated(scores_g[:], gs["mask_p"][:], minval[:])
                nc.vector.tensor_reduce(
                    gs["mx"][:], scores_g[:], axis=AX, op=Alu.max, negate=True
                )
                nc.scalar.activation(
                    attn_g[:], scores_g[:], Exp, bias=gs["mx"][:], scale=1.0,
                    accum_out=gs["sume"][:],
                )
                nc.vector.reciprocal(gs["rsum"][:], gs["sume"][:])
                nc.vector.tensor_scalar_mul(attn_g[:], attn_g[:], gs["rsum"][:])

                # un-permute attn (into the dead scores tile); stored later
                nc.vector.tensor_copy(
                    scores_g[:].rearrange(
                        "b (c p j) -> b c p j", c=_NCH, p=128, j=_NJ
                    ),
                    attn_g[:].rearrange(
                        "b (c j p) -> b c p j", c=_NCH, j=_NJ, p=128
                    ),
                )

            def emit_attn_out(gs, _rep, g):
                nc.gpsimd.dma_start(
                    at_d[g * _GB : (g + 1) * _GB, :], gs["scores"][:]
                )

            def emit_attnT(gs, _rep, g):
                # attn^T columns for the weighted-context matmuls
                for t in range(_NCH * _NJ):
                    tpa = stage_ps.tile(
                        [128, _GB], f32, tag="stage", name=f"tpa{_rep}_{g}_{t}"
                    )
                    nc.tensor.transpose(
                        tpa[:],
                        gs["attn"][:, t * 128 : (t + 1) * 128],
                        ident[0:_GB, 0:_GB],
                    )
                    nc.vector.tensor_copy(
                        gs["attnT"][:, t * _GB : (t + 1) * _GB], tpa[:]
                    )

            pending_wcout = []

            def flush_wcout():
                while pending_wcout:
                    b, wcrow = pending_wcout.pop(0)
                    nc.gpsimd.dma_start(wc_d[b : b + 1, :], wcrow[:])

            def emit_B_batch(gs, _rep, g, bl):
                b = g * _GB + bl
                wcp = wcps_pool.tile(
                    [1, _CTX], f32, tag="wc", name=f"wcp{_rep}_{b}"
                )
                for c in range(_NCH):
                    cx = cxpool.tile(
                        [128, _NJ, _CTX], f32r, tag="cx",
                        name=f"cx{_rep}_{b}_{c}",
                    )
                    nc.gpsimd.dma_start(
                        cx[:],
                        cx_d[b, c * 512 : (c + 1) * 512, :].rearrange(
                            "(p j) d -> p j d", p=128, j=_NJ
                        ),
                    )
                    if c == _NCH - 1:
                        # the previous batch's wc row is ready by now; emitting
                        # it here keeps it from head-of-line-blocking this
                        # batch's context prefetch on the Pool DMA queue
                        flush_wcout()
                    for j in range(_NJ):
                        col = (c * _NJ + j) * _GB + bl
                        for n in range(2):
                            nc.tensor.matmul(
                                wcp[0:1, n * 512 : (n + 1) * 512],
                                gs["attnT"][:, col : col + 1],
                                cx[:, j, n * 512 : (n + 1) * 512],
                                start=(c == 0 and j == 0),
                                stop=(c == _NCH - 1 and j == _NJ - 1),
                            )
                wcrow = rowpool.tile(
                    [1, _CTX], f32, tag="wcrow", name=f"wcrow{_rep}_{b}"
                )
                nc.vector.tensor_copy(wcrow[:], wcp[:])
                pending_wcout.append((b, wcrow))

            # Software pipeline over (rep, group) units:
            #   A(first); smx(first); then per unit: interleave B(prev) with
            #   A(cur) at batch granularity (attnT/attn-out of prev slotted
            #   between batches); smx(cur); finally drain B(last).
            units = [(r, g) for r in range(reps) for g in range(_G)]
            prev = None
            prev_gs = None
            for unit in units:
                r, g = unit
                gs = group_state(r, g)
                for bl in range(_GB):
                    emit_A_batch(gs, r, g, bl)
                    if prev is not None:
                        if bl == 0:
                            emit_attnT(prev_gs, prev[0], prev[1])
                        if bl == 1:
                            emit_attn_out(prev_gs, prev[0], prev[1])
                        emit_B_batch(prev_gs, prev[0], prev[1], bl)
                emit_smx_chain(gs, r, g)
                prev, prev_gs = unit, gs
            emit_attnT(prev_gs, prev[0], prev[1])
            emit_attn_out(prev_gs, prev[0], prev[1])
            for bl in range(_GB):
                emit_B_batch(prev_gs, prev[0], prev[1], bl)
            flush_wcout()

    nc.compile()
    return nc


def _get_nc(reps=1):
    key = ("nc", reps)
    if key not in _CACHE:
        _CACHE[key] = _build_nc(reps)
    return _CACHE[key]


def make_in_maps(**inputs):
    """Shard the full inputs into per-core input maps."""
    h = np.ascontiguousarray(np.asarray(inputs["h"], np.float32))
    pj = np.ascontiguousarray(np.asarray(inputs["proj_context"], np.float32))
    cx = np.ascontiguousarray(np.asarray(inputs["context"], np.float32))
    mk = np.ascontiguousarray(np.asarray(inputs["mask"]).astype(np.uint8))
    W = np.ascontiguousarray(np.asarray(inputs["W_h2attn"], np.float32))
    bh = np.ascontiguousarray(np.asarray(inputs["b_h2attn"], np.float32))
    wa = np.ascontiguousarray(np.asarray(inputs["w_alpha"], np.float32))
    ba = np.asarray(inputs["b_alpha"], np.float32).reshape(1)

    in_maps = []
    for core in range(_NCORES):
        sl = slice(core * _BL, (core + 1) * _BL)
        in_maps.append(
            {
                "h": h[sl],
                "proj_context": pj[sl],
                "context": cx[sl],
                "mask": mk[sl],
                "W_h2attn": W,
                "b_h2attn": bh,
                "w_alpha": wa,
                "b_alpha": ba,
            }
        )
    return in_maps


def kernel(**inputs):
    _ensure_concourse()
    from concourse.bass_utils import run_bass_kernel_spmd

    nc = _get_nc()
    in_maps = make_in_maps(**inputs)
    res = run_bass_kernel_spmd(nc, in_maps, core_ids=list(range(_NCORES))).results

    wc = np.concatenate([res[c]["weighted_context"] for c in range(_NCORES)], axis=0)
    attn = np.concatenate([res[c]["attn"] for c in range(_NCORES)], axis=0)
    return wc, attn
